# revision 9
# baseline (speedup 1.0000x reference)
"""AudioFinder Trainium2 kernel.

Data parallel over batch: 16 samples -> 8 cores x 2 samples.

Per-core pipeline (bf16 matmuls / f32 psum, both samples interleaved
layer-by-layer so one sample's matmuls fill the other's pipeline-latency
bubbles on the in-order engine queues):
  1. Both query encoders (T=2048 -> 504), layers interleaved; v =
     w_lin @ enc_q; the tiled-x4 + pad row [1,2048] is bounced through
     DRAM into [128,16] (t = p + 128*b) while the search encoders run.
  2. Both search encoders (T=8192 -> 2040), layers interleaved.  The
     f0/f1 head + VQ are fused into layer 3's per-chunk pipeline so the
     VQ DVE reductions spread across the conv matmul span.
  3. VQ per 128-t block: ONE bf16 matmul for the scores s[t,k] =
     enc_s[t]@emb[k] - |emb[k]|^2/2 (enc row 80 is const 1.0, epk3 row
     80 carries -|e|^2/2), then on DVE:
       m[t]  = max_k s[t,k]                       (tensor_reduce)
       u_j[t] = max_k (s[t,k] + ew[k,j]/BIG)      (tensor_tensor_reduce
                 against ew-rows replicated across 128 partitions)
       => (u_j - m)*BIG = ew[argmax_k s, j]  with ew = emb @ w_lin.T
  4. z = (u-m)*BIG + vt in [128,16]; max over free dim on DVE, across
     partitions on Pool; out = tanh(max z + b_lin).

Conv layers: 4 taps as PSUM-accumulated matmuls over Cin=80, gated
tanh*sigmoid on ACT engine, gate product on Pool, 1x1 conv + residual
writes on DVE.  wpk is packed layer-major and DMA'd in two pieces so
the first matmul only waits for layer 0's weights.
"""

import numpy as np
import ml_dtypes

import concourse.bacc as bacc
import concourse.mybir as mybir
import concourse.tile as tile
from concourse.bass_utils import run_bass_kernel_spmd

F32 = mybir.dt.float32
BF16 = mybir.dt.bfloat16
AF = mybir.ActivationFunctionType
OP = mybir.AluOpType
AX = mybir.AxisListType

NCORES = 8
USE_TTR = False  # fused tensor_tensor_reduce for the u_j VQ max
SPC = 2          # samples per core
C = 80
NK = 512         # codebook size
BIG = 1024.0
NEG = -1e30
CH = 512         # chunk (free-dim) size

# layer geometry
GEO_SEARCH = dict(T0h=4096, T1=4095, E1=2048, O1=2047, T2=2046, T3=2043, T4=2040)
GEO_QUERY = dict(T0h=1024, T1=1023, E1=512, O1=511, T2=510, T3=507, T4=504)

# wpack layout: layer-major [a_i(4 taps), g_i(4 taps), w1x1_i] x 4, f0, f1
LBLK = 720  # 4*80 + 4*80 + 80

def _w_off(kind, i, j=0):
    if kind == "a":
        return LBLK * i + C * j
    if kind == "g":
        return LBLK * i + 320 + C * j
    if kind == "1":
        return LBLK * i + 640
    if kind == "f0":
        return 2880
    if kind == "f1":
        return 2960
    raise KeyError(kind)


M_F1 = 82  # f1 conv emits 80 real channels + two const-1 channels


WPACK_COLS = 3042
# bias pack columns: ba0..3, bg0..3, b10..3, bf0, bf1
def _b_off(kind, i=0):
    return {"a": i, "g": 4 + i, "1": 8 + i, "f0": 12, "f1": 13}[kind]


def _build():
    nc = bacc.Bacc("TRN2", target_bir_lowering=False, debug=False,
                   num_devices=NCORES)
    d_se = nc.dram_tensor("se", [SPC, C, 4096], BF16, kind="ExternalInput")
    d_so = nc.dram_tensor("so", [SPC, C, 4096], BF16, kind="ExternalInput")
    d_qe = nc.dram_tensor("qe", [SPC, C, 1024], BF16, kind="ExternalInput")
    d_qo = nc.dram_tensor("qo", [SPC, C, 1024], BF16, kind="ExternalInput")
    d_wpk = nc.dram_tensor("wpk", [C, WPACK_COLS], BF16, kind="ExternalInput")
    d_bpk = nc.dram_tensor("bpk", [M_F1, 14], F32, kind="ExternalInput")
    d_epk = nc.dram_tensor("epk", [M_F1, NK], BF16, kind="ExternalInput")
    d_ew = nc.dram_tensor("ew", [2, NK], F32, kind="ExternalInput")
    d_wlt = nc.dram_tensor("wlt", [C, 2], BF16, kind="ExternalInput")
    d_blt = nc.dram_tensor("blt", [1, 4], F32, kind="ExternalInput")
    d_out = nc.dram_tensor("out", [1, 4], F32, kind="ExternalOutput")
    d_zbuf = nc.dram_tensor("zbuf", [2 * SPC, 2048], F32)

    with tile.TileContext(nc) as tc:
        with (
            tc.tile_pool(name="sb", bufs=1) as sb,
            tc.tile_pool(name="ps", bufs=2, space="PSUM") as ps,
        ):
            # ---- startup DMAs, ordered so layer-0 work can start early ----
            wpk = sb.tile([C, WPACK_COLS], BF16, tag="wpk")
            nc.sync.dma_start(wpk[:, :LBLK], d_wpk[:, :LBLK])
            bpk = sb.tile([M_F1, 14], F32, tag="bpk")
            nc.sync.dma_start(bpk[:], d_bpk[:])

            def wsl(kind, i, j=0):
                off = _w_off(kind, i, j)
                return wpk[:, off:off + C]

            def bap(kind, i=0):
                o = _b_off(kind, i)
                n = M_F1 if kind == "f1" else C
                return bpk[:n, o:o + 1]

            def wide_layer(i, taps, T_out, write_out):
                """taps: list of 4 (tile, base_off)."""
                for c0 in range(0, T_out, CH):
                    N = min(CH, T_out - c0)
                    Nmm = N + (N & 1)
                    aps = ps.tile([C, Nmm], F32, tag="aps")
                    gps = ps.tile([C, Nmm], F32, tag="gps")
                    for half, pt in (("a", aps), ("g", gps)):
                        for j, (src, off) in enumerate(taps):
                            nc.tensor.matmul(
                                pt[:], wsl(half, i, j),
                                src[:, off + c0: off + c0 + Nmm],
                                start=(j == 0), stop=(j == 3))
                    ta = sb.tile([C, Nmm], BF16, tag="ta", bufs=2)
                    sg = sb.tile([C, Nmm], BF16, tag="sg", bufs=2)
                    nc.scalar.activation(ta[:], aps[:], AF.Tanh, bias=bap("a", i))
                    nc.scalar.activation(sg[:], gps[:], AF.Sigmoid, bias=bap("g", i))
                    x2 = sb.tile([C, Nmm], BF16, tag="x2", bufs=2)
                    nc.gpsimd.tensor_mul(x2[:], ta[:], sg[:])
                    xps = ps.tile([C, Nmm], F32, tag="xps")
                    nc.tensor.matmul(xps[:], wsl("1", i), x2[:],
                                     start=True, stop=True)
                    write_out(c0, N, xps[:, :N])

            def encoder_steps(s, g, is_query, vq_cb=None):
                """Per-layer thunks; layer 3 fuses the f0/f1 head (+VQ)."""
                T0h, T1 = g["T0h"], g["T1"]
                E1, O1 = g["E1"], g["O1"]
                T2, T3, T4 = g["T2"], g["T3"], g["T4"]
                d_e, d_o = (d_qe, d_qo) if is_query else (d_se, d_so)
                sfx = f"{'q' if is_query else 's'}{s}"
                st = {}

                def load():
                    x0e = sb.tile([C, T0h + 8], BF16, tag=f"x0e{sfx}")
                    x0o = sb.tile([C, T0h + 8], BF16, tag=f"x0o{sfx}")
                    nc.vector.memset(x0e[:, T0h:], 0.0)
                    nc.vector.memset(x0o[:, T0h:], 0.0)
                    for dst, src in ((x0e, d_e), (x0o, d_o)):
                        for c0 in range(0, T0h, 2048):
                            n = min(2048, T0h - c0)
                            nc.sync.dma_start(dst[:, c0:c0 + n],
                                              src[s, :, c0:c0 + n])
                    st["x0e"], st["x0o"] = x0e, x0o

                def l0():
                    x1e = sb.tile([C, E1 + 8], BF16, tag=f"x1e{sfx}")
                    x1o = sb.tile([C, O1 + 8], BF16, tag=f"x1o{sfx}")
                    nc.vector.memset(x1e[:, E1:], 0.0)
                    nc.vector.memset(x1o[:, O1:], 0.0)

                    def w0(c0, N, xps):
                        ne, no = (N + 1) // 2, N // 2
                        h = c0 // 2
                        nc.vector.tensor_scalar(
                            x1e[:, h:h + ne], xps[:, 0:N:2], bap("1", 0),
                            None, op0=OP.add)
                        nc.vector.tensor_scalar(
                            x1o[:, h:h + no], xps[:, 1:N:2], bap("1", 0),
                            None, op0=OP.add)

                    wide_layer(0, [(st["x0e"], 0), (st["x0o"], 0),
                                   (st["x0e"], 1), (st["x0o"], 1)], T1, w0)
                    st["x1e"], st["x1o"] = x1e, x1o

                def l1():
                    x2f = sb.tile([C, T2 + 8], BF16, tag=f"x2f{sfx}")
                    nc.vector.memset(x2f[:, T2:], 0.0)

                    def w1(c0, N, xps):
                        nc.vector.scalar_tensor_tensor(
                            out=x2f[:, c0:c0 + N], in0=xps, scalar=bap("1", 1),
                            in1=st["x1o"][:, c0 + 1:c0 + 1 + N],
                            op0=OP.add, op1=OP.add)

                    wide_layer(1, [(st["x1e"], 0), (st["x1o"], 0),
                                   (st["x1e"], 1), (st["x1o"], 1)], T2, w1)
                    st["x2f"] = x2f

                def l2():
                    x3f = sb.tile([C, T3 + 8], BF16, tag=f"x3f{sfx}")
                    nc.vector.memset(x3f[:, T3:], 0.0)

                    def w2(c0, N, xps):
                        nc.vector.scalar_tensor_tensor(
                            out=x3f[:, c0:c0 + N], in0=xps, scalar=bap("1", 2),
                            in1=st["x2f"][:, c0 + 3:c0 + 3 + N],
                            op0=OP.add, op1=OP.add)

                    wide_layer(2, [(st["x2f"], 0), (st["x2f"], 1),
                                   (st["x2f"], 2), (st["x2f"], 3)], T3, w2)
                    st["x3f"] = x3f

                def l3():
                    x4f = sb.tile([C, T4 + 8], BF16, tag=f"x4f{sfx}")
                    nc.vector.memset(x4f[:, T4:], 0.0)
                    # f1(relu(f0(x))); rows 80/81 of enc are const 1.0
                    # (zero weights, bias 1) for the VQ score offsets.
                    enc = sb.tile([M_F1, T4], BF16, tag=f"enc{sfx}")

                    def w3(c0, N, xps):
                        nc.vector.scalar_tensor_tensor(
                            out=x4f[:, c0:c0 + N], in0=xps, scalar=bap("1", 3),
                            in1=st["x3f"][:, c0 + 3:c0 + 3 + N],
                            op0=OP.add, op1=OP.add)
                        p0 = ps.tile([C, N], F32, tag="xps")
                        nc.tensor.matmul(p0[:], wsl("f0", 0),
                                         x4f[:, c0:c0 + N],
                                         start=True, stop=True)
                        xf = sb.tile([C, N], BF16, tag="xf", bufs=2)
                        nc.scalar.activation(xf[:], p0[:], AF.Relu,
                                             bias=bap("f0"))
                        p1 = ps.tile([M_F1, N], F32, tag="xps")
                        nc.tensor.matmul(p1[:], wpk[:, 2960:2960 + M_F1],
                                         xf[:], start=True, stop=True)
                        nc.scalar.activation(enc[:, c0:c0 + N], p1[:],
                                             AF.Identity, bias=bap("f1"))
                        if vq_cb is not None:
                            vq_cb(c0, N, enc)

                    wide_layer(3, [(st["x3f"], 0), (st["x3f"], 1),
                                   (st["x3f"], 2), (st["x3f"], 3)], T4, w3)
                    st["enc"] = enc

                return [load, l0, l1, l2, l3], st

            # ---- query encoders, interleaved ----
            qsteps = [encoder_steps(s, GEO_QUERY, True) for s in range(SPC)]
            for q in qsteps:
                q[0][0]()          # both loads first (DMA queue order)
            # remaining static tables can arrive behind the query inputs
            nc.sync.dma_start(wpk[:, LBLK:], d_wpk[:, LBLK:])
            epk = sb.tile([M_F1, NK], BF16, tag="epk")
            nc.sync.dma_start(epk[:], d_epk[:])
            ewsb = []
            for j in range(2):
                ewr = sb.tile([1, NK], F32, tag=f"ewsb{j}")
                nc.sync.dma_start(ewr[:], d_ew[j:j + 1, :])
                ewsb.append(ewr)
            wlt = sb.tile([C, 2], BF16, tag="wlt")
            nc.sync.dma_start(wlt[:], d_wlt[:])
            brow = sb.tile([1, 4], F32, tag="brow")
            nc.sync.dma_start(brow[:], d_blt[:])

            for pair in zip(*[q[0][1:] for q in qsteps]):
                for fn in pair:
                    fn()

            # replicate ew/BIG rows across 128 partitions (ones-matmul)
            ones = sb.tile([1, 128], F32, tag="ones")
            nc.vector.memset(ones[:], 1.0)
            ereps = []
            for j in range(2):
                pj = ps.tile([128, NK], F32, tag="vq")
                nc.tensor.matmul(pj[:], ones[:], ewsb[j][:],
                                 start=True, stop=True)
                erep = sb.tile([128, NK], F32, tag=f"erep{j}")
                nc.vector.tensor_copy(erep[:], pj[:])
                ereps.append(erep)

            zred = sb.tile([128, 4], F32, tag="zred")
            vts = {}
            for s in range(SPC):
                enc_q = qsteps[s][1]["enc"]
                vps = ps.tile([2, 504], F32, tag="xps")
                nc.tensor.matmul(vps[:], wlt[:], enc_q[:C, :504],
                                 start=True, stop=True)
                vrow = sb.tile([2, 2048], F32, tag="vrow", bufs=2)
                for k in range(4):
                    nc.vector.tensor_copy(vrow[:, 504 * k:504 * (k + 1)],
                                          vps[:])
                nc.vector.memset(vrow[:, 2016:2040], 0.0)
                nc.vector.memset(vrow[:, 2040:2048], NEG)
                nc.sync.dma_start(d_zbuf[2 * s:2 * s + 2, :], vrow[:])
                vt0 = sb.tile([128, 16], F32, tag=f"vt{2 * s}")
                vt1 = sb.tile([128, 16], F32, tag=f"vt{2 * s + 1}")
                nc.sync.dma_start(
                    vt0[:], d_zbuf[2 * s].rearrange("(b p) -> p b", p=128))
                nc.sync.dma_start(
                    vt1[:], d_zbuf[2 * s + 1].rearrange("(b p) -> p b", p=128))
                vts[s] = (vt0, vt1)

            # ---- search encoders with fused VQ, interleaved ----
            T4s = GEO_SEARCH["T4"]
            accs = {}
            for s in range(SPC):
                mt = sb.tile([128, 16], F32, tag=f"mt{s}")
                u0t = sb.tile([128, 16], F32, tag=f"u0t{s}")
                u1t = sb.tile([128, 16], F32, tag=f"u1t{s}")
                nc.vector.memset(mt[:], NEG)
                nc.vector.memset(u0t[:], NEG)
                nc.vector.memset(u1t[:], NEG)
                accs[s] = (mt, u0t, u1t)

            def make_vq_cb(s):
                mt, u0t, u1t = accs[s]

                def vq_blocks(c0, N, enc):
                    b0 = (c0 + 127) // 128
                    b1 = (c0 + N) // 128 if c0 + N < T4s else 16
                    for b in range(b0, b1):
                        t0 = 128 * b
                        P = min(128, T4s - t0)
                        sps = ps.tile([P, NK], F32, tag="vq")
                        nc.tensor.matmul(sps[:], enc[:, t0:t0 + P],
                                         epk[:, :], start=True, stop=True)
                        nc.vector.tensor_reduce(
                            mt[:P, b:b + 1], sps[:], axis=AX.X, op=OP.max)
                        for ut, erep in ((u0t, ereps[0]), (u1t, ereps[1])):
                            scr = sb.tile([P, NK], F32, tag="vqscr", bufs=2)
                            if USE_TTR:
                                nc.vector.tensor_tensor_reduce(
                                    out=scr[:], in0=sps[:], in1=erep[:P, :],
                                    scale=1.0, scalar=NEG, op0=OP.add,
                                    op1=OP.max, accum_out=ut[:P, b:b + 1])
                            else:
                                nc.vector.tensor_add(scr[:], sps[:],
                                                     erep[:P, :])
                                nc.vector.tensor_reduce(
                                    ut[:P, b:b + 1], scr[:], axis=AX.X,
                                    op=OP.max)
                return vq_blocks

            ssteps = [encoder_steps(s, GEO_SEARCH, False,
                                    vq_cb=make_vq_cb(s))
                      for s in range(SPC)]
            for pair in zip(*[q[0] for q in ssteps]):
                for fn in pair:
                    fn()

            # ---- z = (u - m)*BIG + vt, reduce ----
            for s in range(SPC):
                mt, u0t, u1t = accs[s]
                for j, ut in ((0, u0t), (1, u1t)):
                    zt = sb.tile([128, 16], F32, tag="zt", bufs=2)
                    nc.vector.tensor_sub(zt[:], ut[:], mt[:])
                    nc.vector.scalar_tensor_tensor(
                        out=zt[:], in0=zt[:], scalar=BIG, in1=vts[s][j][:],
                        op0=OP.mult, op1=OP.add)
                    nc.vector.tensor_reduce(
                        zred[:, 2 * s + j:2 * s + j + 1], zt[:], axis=AX.X,
                        op=OP.max)

            zrow = sb.tile([1, 4], F32, tag="zrow")
            nc.gpsimd.tensor_reduce(zrow[:], zred[:], axis=AX.C, op=OP.max)
            nc.vector.tensor_add(zrow[:], zrow[:], brow[:])
            outv = sb.tile([1, 4], F32, tag="outv")
            nc.scalar.activation(outv[:], zrow[:], AF.Tanh)
            nc.sync.dma_start(d_out[:], outv[:])

    nc.finalize()
    return nc


_NC_CACHE = None


def _get_nc():
    global _NC_CACHE
    if _NC_CACHE is None:
        _NC_CACHE = _build()
    return _NC_CACHE


def prep_inputs(search, query, w_wide, b_wide, w_1x1, b_1x1, w_f0, b_f0,
                w_f1, b_f1, embedding, w_lin, b_lin):
    """Host-side packing -> list of per-core input maps (bf16 operands)."""
    f = np.float32
    bf = ml_dtypes.bfloat16
    search = np.asarray(search, f)
    query = np.asarray(query, f)
    se = np.ascontiguousarray(search[:, 0::2, :].transpose(0, 2, 1)).astype(bf)
    so = np.ascontiguousarray(search[:, 1::2, :].transpose(0, 2, 1)).astype(bf)
    qe = np.ascontiguousarray(query[:, 0::2, :].transpose(0, 2, 1)).astype(bf)
    qo = np.ascontiguousarray(query[:, 1::2, :].transpose(0, 2, 1)).astype(bf)

    w_wide = np.asarray(w_wide, f)
    cols = []
    for i in range(4):
        for j in range(4):
            cols.append(w_wide[i, :C, :, j].T)     # a taps
        for j in range(4):
            cols.append(w_wide[i, C:, :, j].T)     # g taps
        cols.append(np.asarray(w_1x1, f)[i, :, :, 0].T)
    cols.append(np.asarray(w_f0, f)[:, :, 0].T)
    wf1 = np.zeros((C, M_F1), f)
    wf1[:, :C] = np.asarray(w_f1, f)[:, :, 0].T   # cols 80/81 stay zero
    cols.append(wf1)
    wpk = np.ascontiguousarray(np.concatenate(cols, axis=1)).astype(bf)
    assert wpk.shape == (C, WPACK_COLS)

    b_wide = np.asarray(b_wide, f)
    bcols = [b_wide[i, :C] for i in range(4)]
    bcols += [b_wide[i, C:] for i in range(4)]
    bcols += [np.asarray(b_1x1, f)[i] for i in range(4)]
    bcols += [np.asarray(b_f0, f), np.asarray(b_f1, f)]
    bpk = np.zeros((M_F1, 14), f)
    bpk[:C] = np.stack(bcols, axis=1)
    bpk[C, _b_off("f1")] = 1.0     # f1 rows 80/81 = 0*x + 1.0 -> const-1
    bpk[C + 1, _b_off("f1")] = 1.0

    emb = np.asarray(embedding, f)[0]            # (512, 80)
    e2 = (emb.astype(np.float64) ** 2).sum(1)
    ew = (emb.astype(np.float64) @ np.asarray(w_lin, f).T.astype(np.float64))
    epk = np.zeros((M_F1, NK), f)
    epk[:C] = emb.T
    epk[C] = -0.5 * e2
    epk = epk.astype(bf)
    ewp = np.ascontiguousarray((ew.T / BIG).astype(f))   # (2, NK)
    wlt = np.ascontiguousarray(np.asarray(w_lin, f).T).astype(bf)
    b_lin = np.asarray(b_lin, f)
    blt = np.array([[b_lin[0], b_lin[1], b_lin[0], b_lin[1]]], f)

    maps = []
    for c in range(NCORES):
        sl = slice(SPC * c, SPC * (c + 1))
        maps.append({
            "se": se[sl], "so": so[sl], "qe": qe[sl], "qo": qo[sl],
            "wpk": wpk, "bpk": bpk, "epk": epk, "ew": ewp, "wlt": wlt,
            "blt": blt,
        })
    return maps


def kernel(**inputs):
    nc = _get_nc()
    maps = prep_inputs(**inputs)
    res = run_bass_kernel_spmd(nc, maps, core_ids=list(range(NCORES)))
    out = np.concatenate([r["out"].reshape(SPC, 2) for r in res.results],
                         axis=0)
    return out.astype(np.float32)


if __name__ == "__main__":
    import reference
    inputs = {k: np.asarray(v) for k, v in reference.setup_inputs().items()}
    got = kernel(**inputs)
    print(got)


# revision 16
# speedup vs baseline: 1.1689x; 1.1689x over previous
"""AudioFinder Trainium2 kernel.

Data parallel over batch: 16 samples -> 8 cores x 2 samples.

Per-core pipeline (bf16 matmuls / f32 psum, both samples interleaved
layer-by-layer so one sample's matmuls fill the other's pipeline-latency
bubbles on the in-order engine queues):
  1. Both query encoders (T=2048 -> 504), layers interleaved; v =
     w_lin @ enc_q; the tiled-x4 + pad row [1,2048] is bounced through
     DRAM into [128,16] (t = p + 128*b) while the search encoders run.
  2. Both search encoders (T=8192 -> 2040), layers interleaved.  The
     f0/f1 head + VQ are fused into layer 3's per-chunk pipeline so the
     VQ DVE reductions spread across the conv matmul span.
  3. VQ per 128-t block: three bf16 matmuls (enc block stationary)
     against epk3's three NK-column blocks, DVE max-reduce each:
     scores s[t,k] = enc_s[t]@emb[k] - |emb[k]|^2/2 (enc rows 80/81
     const 1.0; epk3 row 80 carries -|e|^2/2 shared, row 81 carries
     0 / ew0/BIG / ew1/BIG with ew = emb @ w_lin.T):
       u_j[t] = max_k (s[t,k] + ew[k,j]/BIG),  m[t] = max_k s[t,k]
       => (u_j - m)*BIG = ew[argmax_k s, j]   (fp32-psum exact)
  4. z = (u-m)*BIG + vt in [128,16]; max over free dim on DVE, across
     partitions on Pool; out = tanh(max z + b_lin).

Conv layers: 4 taps as PSUM-accumulated matmuls over Cin=80, gated
tanh*sigmoid on ACT engine, gate product on Pool, 1x1 conv + residual
writes on DVE.  wpk is packed layer-major and DMA'd in two pieces so
the first matmul only waits for layer 0's weights.
"""

import numpy as np
import ml_dtypes

import concourse.bacc as bacc
import concourse.mybir as mybir
import concourse.tile as tile
from concourse.bass_utils import run_bass_kernel_spmd

F32 = mybir.dt.float32
BF16 = mybir.dt.bfloat16
AF = mybir.ActivationFunctionType
OP = mybir.AluOpType
AX = mybir.AxisListType

NCORES = 8
SPC = 2          # samples per core
C = 80
NK = 512         # codebook size
BIG = 1024.0
NEG = -1e30
CH = 512         # chunk (free-dim) size

# layer geometry
GEO_SEARCH = dict(T0h=4096, T1=4095, E1=2048, O1=2047, T2=2046, T3=2043, T4=2040)
GEO_QUERY = dict(T0h=1024, T1=1023, E1=512, O1=511, T2=510, T3=507, T4=504)

# wpack layout: layer-major [a_i(4 taps), g_i(4 taps), w1x1_i] x 4, f0, f1
LBLK = 720  # 4*80 + 4*80 + 80

def _w_off(kind, i, j=0):
    if kind == "a":
        return LBLK * i + C * j
    if kind == "g":
        return LBLK * i + 320 + C * j
    if kind == "1":
        return LBLK * i + 640
    if kind == "f0":
        return 2880
    if kind == "f1":
        return 2960
    raise KeyError(kind)


M_F1 = 82  # f1 conv emits 80 real channels + two const-1 channels


WPACK_COLS = 3042
# bias pack columns: ba0..3, bg0..3, b10..3, bf0, bf1
def _b_off(kind, i=0):
    return {"a": i, "g": 4 + i, "1": 8 + i, "f0": 12, "f1": 13}[kind]


def _build():
    nc = bacc.Bacc("TRN2", target_bir_lowering=False, debug=False,
                   num_devices=NCORES)
    d_se = nc.dram_tensor("se", [SPC, C, 4096], BF16, kind="ExternalInput")
    d_so = nc.dram_tensor("so", [SPC, C, 4096], BF16, kind="ExternalInput")
    d_qe = nc.dram_tensor("qe", [SPC, C, 1024], BF16, kind="ExternalInput")
    d_qo = nc.dram_tensor("qo", [SPC, C, 1024], BF16, kind="ExternalInput")
    d_wpk = nc.dram_tensor("wpk", [C, WPACK_COLS], BF16, kind="ExternalInput")
    d_bpk = nc.dram_tensor("bpk", [M_F1, 14], F32, kind="ExternalInput")
    d_epk3 = nc.dram_tensor("epk3", [M_F1, 3 * NK], BF16, kind="ExternalInput")
    d_wlt = nc.dram_tensor("wlt", [C, 2], BF16, kind="ExternalInput")
    d_blt = nc.dram_tensor("blt", [1, 4], F32, kind="ExternalInput")
    d_out = nc.dram_tensor("out", [1, 4], F32, kind="ExternalOutput")
    d_zbuf = nc.dram_tensor("zbuf", [2 * SPC, 2048], F32)

    with tile.TileContext(nc) as tc:
        with (
            tc.tile_pool(name="sb", bufs=1) as sb,
            tc.tile_pool(name="ps", bufs=2, space="PSUM") as ps,
        ):
            # ---- startup DMAs, ordered so layer-0 work can start early ----
            wpk = sb.tile([C, WPACK_COLS], BF16, tag="wpk")
            nc.sync.dma_start(wpk[:, :LBLK], d_wpk[:, :LBLK])
            bpk = sb.tile([M_F1, 14], F32, tag="bpk")
            nc.sync.dma_start(bpk[:], d_bpk[:])

            def wsl(kind, i, j=0):
                off = _w_off(kind, i, j)
                return wpk[:, off:off + C]

            def bap(kind, i=0):
                o = _b_off(kind, i)
                n = M_F1 if kind == "f1" else C
                return bpk[:n, o:o + 1]

            def wide_layer(i, taps, T_out, write_out):
                """taps: list of 4 (tile, base_off)."""
                for c0 in range(0, T_out, CH):
                    N = min(CH, T_out - c0)
                    Nmm = N + (N & 1)
                    aps = ps.tile([C, Nmm], F32, tag="aps")
                    gps = ps.tile([C, Nmm], F32, tag="gps")
                    for half, pt in (("a", aps), ("g", gps)):
                        for j, (src, off) in enumerate(taps):
                            nc.tensor.matmul(
                                pt[:], wsl(half, i, j),
                                src[:, off + c0: off + c0 + Nmm],
                                start=(j == 0), stop=(j == 3))
                    ta = sb.tile([C, Nmm], BF16, tag="ta", bufs=2)
                    sg = sb.tile([C, Nmm], BF16, tag="sg", bufs=2)
                    nc.scalar.activation(ta[:], aps[:], AF.Tanh, bias=bap("a", i))
                    nc.scalar.activation(sg[:], gps[:], AF.Sigmoid, bias=bap("g", i))
                    x2 = sb.tile([C, Nmm], BF16, tag="x2", bufs=2)
                    nc.gpsimd.tensor_mul(x2[:], ta[:], sg[:])
                    xps = ps.tile([C, Nmm], F32, tag="xps")
                    nc.tensor.matmul(xps[:], wsl("1", i), x2[:],
                                     start=True, stop=True)
                    write_out(c0, N, xps[:, :N])

            def encoder_steps(s, g, is_query, vq_cb=None):
                """Per-layer thunks; layer 3 fuses the f0/f1 head (+VQ)."""
                T0h, T1 = g["T0h"], g["T1"]
                E1, O1 = g["E1"], g["O1"]
                T2, T3, T4 = g["T2"], g["T3"], g["T4"]
                d_e, d_o = (d_qe, d_qo) if is_query else (d_se, d_so)
                sfx = f"{'q' if is_query else 's'}{s}"
                st = {}

                def load():
                    x0e = sb.tile([C, T0h + 8], BF16, tag=f"x0e{sfx}")
                    x0o = sb.tile([C, T0h + 8], BF16, tag=f"x0o{sfx}")
                    nc.vector.memset(x0e[:, T0h:], 0.0)
                    nc.vector.memset(x0o[:, T0h:], 0.0)
                    for dst, src in ((x0e, d_e), (x0o, d_o)):
                        for c0 in range(0, T0h, 2048):
                            n = min(2048, T0h - c0)
                            nc.sync.dma_start(dst[:, c0:c0 + n],
                                              src[s, :, c0:c0 + n])
                    st["x0e"], st["x0o"] = x0e, x0o

                def l0():
                    x1e = sb.tile([C, E1 + 8], BF16, tag=f"x1e{sfx}")
                    x1o = sb.tile([C, O1 + 8], BF16, tag=f"x1o{sfx}")
                    nc.vector.memset(x1e[:, E1:], 0.0)
                    nc.vector.memset(x1o[:, O1:], 0.0)

                    def w0(c0, N, xps):
                        ne, no = (N + 1) // 2, N // 2
                        h = c0 // 2
                        nc.vector.tensor_scalar(
                            x1e[:, h:h + ne], xps[:, 0:N:2], bap("1", 0),
                            None, op0=OP.add)
                        nc.vector.tensor_scalar(
                            x1o[:, h:h + no], xps[:, 1:N:2], bap("1", 0),
                            None, op0=OP.add)

                    wide_layer(0, [(st["x0e"], 0), (st["x0o"], 0),
                                   (st["x0e"], 1), (st["x0o"], 1)], T1, w0)
                    st["x1e"], st["x1o"] = x1e, x1o

                def l1():
                    x2f = sb.tile([C, T2 + 8], BF16, tag=f"x2f{sfx}")
                    nc.vector.memset(x2f[:, T2:], 0.0)

                    def w1(c0, N, xps):
                        nc.vector.scalar_tensor_tensor(
                            out=x2f[:, c0:c0 + N], in0=xps, scalar=bap("1", 1),
                            in1=st["x1o"][:, c0 + 1:c0 + 1 + N],
                            op0=OP.add, op1=OP.add)

                    wide_layer(1, [(st["x1e"], 0), (st["x1o"], 0),
                                   (st["x1e"], 1), (st["x1o"], 1)], T2, w1)
                    st["x2f"] = x2f

                def l2():
                    x3f = sb.tile([C, T3 + 8], BF16, tag=f"x3f{sfx}")
                    nc.vector.memset(x3f[:, T3:], 0.0)

                    def w2(c0, N, xps):
                        nc.vector.scalar_tensor_tensor(
                            out=x3f[:, c0:c0 + N], in0=xps, scalar=bap("1", 2),
                            in1=st["x2f"][:, c0 + 3:c0 + 3 + N],
                            op0=OP.add, op1=OP.add)

                    wide_layer(2, [(st["x2f"], 0), (st["x2f"], 1),
                                   (st["x2f"], 2), (st["x2f"], 3)], T3, w2)
                    st["x3f"] = x3f

                def l3():
                    x4f = sb.tile([C, T4 + 8], BF16, tag=f"x4f{sfx}")
                    nc.vector.memset(x4f[:, T4:], 0.0)
                    # f1(relu(f0(x))); rows 80/81 of enc are const 1.0
                    # (zero weights, bias 1) for the VQ score offsets.
                    enc = sb.tile([M_F1, T4], BF16, tag=f"enc{sfx}")

                    def w3(c0, N, xps):
                        nc.vector.scalar_tensor_tensor(
                            out=x4f[:, c0:c0 + N], in0=xps, scalar=bap("1", 3),
                            in1=st["x3f"][:, c0 + 3:c0 + 3 + N],
                            op0=OP.add, op1=OP.add)
                        p0 = ps.tile([C, N], F32, tag="xps")
                        nc.tensor.matmul(p0[:], wsl("f0", 0),
                                         x4f[:, c0:c0 + N],
                                         start=True, stop=True)
                        xf = sb.tile([C, N], BF16, tag="xf", bufs=2)
                        nc.scalar.activation(xf[:], p0[:], AF.Relu,
                                             bias=bap("f0"))
                        p1 = ps.tile([M_F1, N], F32, tag="xps")
                        nc.tensor.matmul(p1[:], wpk[:, 2960:2960 + M_F1],
                                         xf[:], start=True, stop=True)
                        nc.scalar.activation(enc[:, c0:c0 + N], p1[:],
                                             AF.Identity, bias=bap("f1"))
                        if vq_cb is not None:
                            vq_cb(c0, N, enc)

                    wide_layer(3, [(st["x3f"], 0), (st["x3f"], 1),
                                   (st["x3f"], 2), (st["x3f"], 3)], T4, w3)
                    st["enc"] = enc

                return [load, l0, l1, l2, l3], st

            # ---- query encoders, interleaved ----
            qsteps = [encoder_steps(s, GEO_QUERY, True) for s in range(SPC)]
            for q in qsteps:
                q[0][0]()          # both loads first (DMA queue order)
            # remaining static tables can arrive behind the query inputs
            nc.sync.dma_start(wpk[:, LBLK:], d_wpk[:, LBLK:])
            epk3 = sb.tile([M_F1, 3 * NK], BF16, tag="epk3")
            nc.sync.dma_start(epk3[:], d_epk3[:])
            wlt = sb.tile([C, 2], BF16, tag="wlt")
            nc.sync.dma_start(wlt[:], d_wlt[:])
            brow = sb.tile([1, 4], F32, tag="brow")
            nc.sync.dma_start(brow[:], d_blt[:])

            for pair in zip(*[q[0][1:] for q in qsteps]):
                for fn in pair:
                    fn()

            zred = sb.tile([128, 4], F32, tag="zred")
            vts = {}
            for s in range(SPC):
                enc_q = qsteps[s][1]["enc"]
                vps = ps.tile([2, 504], F32, tag="xps")
                nc.tensor.matmul(vps[:], wlt[:], enc_q[:C, :504],
                                 start=True, stop=True)
                vrow = sb.tile([2, 2048], F32, tag="vrow", bufs=2)
                for k in range(4):
                    nc.vector.tensor_copy(vrow[:, 504 * k:504 * (k + 1)],
                                          vps[:])
                nc.vector.memset(vrow[:, 2016:2040], 0.0)
                nc.vector.memset(vrow[:, 2040:2048], NEG)
                nc.sync.dma_start(d_zbuf[2 * s:2 * s + 2, :], vrow[:])
                vt0 = sb.tile([128, 16], F32, tag=f"vt{2 * s}")
                vt1 = sb.tile([128, 16], F32, tag=f"vt{2 * s + 1}")
                nc.sync.dma_start(
                    vt0[:], d_zbuf[2 * s].rearrange("(b p) -> p b", p=128))
                nc.sync.dma_start(
                    vt1[:], d_zbuf[2 * s + 1].rearrange("(b p) -> p b", p=128))
                vts[s] = (vt0, vt1)

            # ---- search encoders with fused VQ, interleaved ----
            T4s = GEO_SEARCH["T4"]
            accs = {}
            for s in range(SPC):
                mt = sb.tile([128, 16], F32, tag=f"mt{s}")
                u0t = sb.tile([128, 16], F32, tag=f"u0t{s}")
                u1t = sb.tile([128, 16], F32, tag=f"u1t{s}")
                nc.vector.memset(mt[:], NEG)
                nc.vector.memset(u0t[:], NEG)
                nc.vector.memset(u1t[:], NEG)
                accs[s] = (mt, u0t, u1t)

            def make_vq_cb(s):
                mt, u0t, u1t = accs[s]

                def vq_blocks(c0, N, enc):
                    b0 = (c0 + 127) // 128
                    b1 = (c0 + N) // 128 if c0 + N < T4s else 16
                    for b in range(b0, b1):
                        t0 = 128 * b
                        P = min(128, T4s - t0)
                        for ti, tgt in ((0, mt), (1, u0t), (2, u1t)):
                            sps = ps.tile([P, NK], F32, tag="vq")
                            nc.tensor.matmul(
                                sps[:], enc[:, t0:t0 + P],
                                epk3[:, NK * ti:NK * (ti + 1)],
                                start=True, stop=True)
                            nc.vector.tensor_reduce(
                                tgt[:P, b:b + 1], sps[:], axis=AX.X,
                                op=OP.max)
                return vq_blocks

            ssteps = [encoder_steps(s, GEO_SEARCH, False,
                                    vq_cb=make_vq_cb(s))
                      for s in range(SPC)]
            for pair in zip(*[q[0] for q in ssteps]):
                for fn in pair:
                    fn()

            # ---- z = (u - m)*BIG + vt, reduce ----
            for s in range(SPC):
                mt, u0t, u1t = accs[s]
                for j, ut in ((0, u0t), (1, u1t)):
                    zt = sb.tile([128, 16], F32, tag="zt", bufs=2)
                    nc.vector.tensor_sub(zt[:], ut[:], mt[:])
                    nc.vector.scalar_tensor_tensor(
                        out=zt[:], in0=zt[:], scalar=BIG, in1=vts[s][j][:],
                        op0=OP.mult, op1=OP.add)
                    nc.vector.tensor_reduce(
                        zred[:, 2 * s + j:2 * s + j + 1], zt[:], axis=AX.X,
                        op=OP.max)

            zrow = sb.tile([1, 4], F32, tag="zrow")
            nc.gpsimd.tensor_reduce(zrow[:], zred[:], axis=AX.C, op=OP.max)
            nc.vector.tensor_add(zrow[:], zrow[:], brow[:])
            outv = sb.tile([1, 4], F32, tag="outv")
            nc.scalar.activation(outv[:], zrow[:], AF.Tanh)
            nc.sync.dma_start(d_out[:], outv[:])

    nc.finalize()
    return nc


_NC_CACHE = None


def _get_nc():
    global _NC_CACHE
    if _NC_CACHE is None:
        _NC_CACHE = _build()
    return _NC_CACHE


def prep_inputs(search, query, w_wide, b_wide, w_1x1, b_1x1, w_f0, b_f0,
                w_f1, b_f1, embedding, w_lin, b_lin):
    """Host-side packing -> list of per-core input maps (bf16 operands)."""
    f = np.float32
    bf = ml_dtypes.bfloat16
    search = np.asarray(search, f)
    query = np.asarray(query, f)
    se = np.ascontiguousarray(search[:, 0::2, :].transpose(0, 2, 1)).astype(bf)
    so = np.ascontiguousarray(search[:, 1::2, :].transpose(0, 2, 1)).astype(bf)
    qe = np.ascontiguousarray(query[:, 0::2, :].transpose(0, 2, 1)).astype(bf)
    qo = np.ascontiguousarray(query[:, 1::2, :].transpose(0, 2, 1)).astype(bf)

    w_wide = np.asarray(w_wide, f)
    cols = []
    for i in range(4):
        for j in range(4):
            cols.append(w_wide[i, :C, :, j].T)     # a taps
        for j in range(4):
            cols.append(w_wide[i, C:, :, j].T)     # g taps
        cols.append(np.asarray(w_1x1, f)[i, :, :, 0].T)
    cols.append(np.asarray(w_f0, f)[:, :, 0].T)
    wf1 = np.zeros((C, M_F1), f)
    wf1[:, :C] = np.asarray(w_f1, f)[:, :, 0].T   # cols 80/81 stay zero
    cols.append(wf1)
    wpk = np.ascontiguousarray(np.concatenate(cols, axis=1)).astype(bf)
    assert wpk.shape == (C, WPACK_COLS)

    b_wide = np.asarray(b_wide, f)
    bcols = [b_wide[i, :C] for i in range(4)]
    bcols += [b_wide[i, C:] for i in range(4)]
    bcols += [np.asarray(b_1x1, f)[i] for i in range(4)]
    bcols += [np.asarray(b_f0, f), np.asarray(b_f1, f)]
    bpk = np.zeros((M_F1, 14), f)
    bpk[:C] = np.stack(bcols, axis=1)
    bpk[C, _b_off("f1")] = 1.0     # f1 rows 80/81 = 0*x + 1.0 -> const-1
    bpk[C + 1, _b_off("f1")] = 1.0

    emb = np.asarray(embedding, f)[0]            # (512, 80)
    e2 = (emb.astype(np.float64) ** 2).sum(1)
    ew = (emb.astype(np.float64) @ np.asarray(w_lin, f).T.astype(np.float64))
    epk3 = np.zeros((M_F1, 3 * NK), f)
    for ti in range(3):
        epk3[:C, NK * ti:NK * (ti + 1)] = emb.T
        epk3[C, NK * ti:NK * (ti + 1)] = -0.5 * e2
    epk3[C + 1, NK:2 * NK] = ew[:, 0] / BIG
    epk3[C + 1, 2 * NK:3 * NK] = ew[:, 1] / BIG
    epk3 = epk3.astype(bf)
    wlt = np.ascontiguousarray(np.asarray(w_lin, f).T).astype(bf)
    b_lin = np.asarray(b_lin, f)
    blt = np.array([[b_lin[0], b_lin[1], b_lin[0], b_lin[1]]], f)

    maps = []
    for c in range(NCORES):
        sl = slice(SPC * c, SPC * (c + 1))
        maps.append({
            "se": se[sl], "so": so[sl], "qe": qe[sl], "qo": qo[sl],
            "wpk": wpk, "bpk": bpk, "epk3": epk3, "wlt": wlt, "blt": blt,
        })
    return maps


def kernel(**inputs):
    nc = _get_nc()
    maps = prep_inputs(**inputs)
    res = run_bass_kernel_spmd(nc, maps, core_ids=list(range(NCORES)))
    out = np.concatenate([r["out"].reshape(SPC, 2) for r in res.results],
                         axis=0)
    return out.astype(np.float32)


if __name__ == "__main__":
    import reference
    inputs = {k: np.asarray(v) for k, v in reference.setup_inputs().items()}
    got = kernel(**inputs)
    print(got)
